# revision 4
# baseline (speedup 1.0000x reference)
"""Trainium2 Bass kernel for nn_Attention (sparse attention with seq_sizes masking).

Computation (per example b over its own T axis):
    query = lrelu(dec @ Wq.T + bq)                        [B, 128]
    key   = lrelu(LF @ Wk.T + bk)                         [B, T, 128]
    energy= einsum('bk,btk->bt', query, key)              [B, T]
    att   = softmax(energy) * mask;  att /= sum(att)      [B, T]
    value = lrelu(LF @ Wv.T + bv)                         [B, T, 128]
    ctx   = einsum('bt,btc->bc', att, value)              [B, 128]

Strategy: data-parallel over B across 8 cores. Because att[t>=seq]==0 exactly and
the pre-mask softmax normalization cancels, rows t >= seq_sizes[b] never matter.
The host packs only the valid (128-rounded) row-range of each example into a
dense per-core buffer (LPT-balanced), pre-transposed to [f, t] layout so the
contraction dim lands on SBUF partitions. On device, per 512-column batch:
fp32r matmuls produce keyT/valueT, Prelu(+bias) applies the activation from
PSUM, energy = queryT.T @ keyT, exp on ACT, masked-sum via one fused DVE
tensor_tensor_reduce, and the context accumulates *unnormalized* scores via PE
(score-chunk transposes + [8,128] accumulating matmuls); normalization is a
per-partition scale folded into the final PSUM->SBUF copy.
"""

import numpy as np
import ml_dtypes

import concourse.bass as bass
import concourse.mybir as mybir
from concourse import bacc
from concourse.tile import TileContext
from concourse.masks import make_identity
from concourse.bass_utils import run_bass_kernel_spmd

F32 = mybir.dt.float32
F32R = mybir.dt.float32r
BF16 = mybir.dt.bfloat16
AF = mybir.ActivationFunctionType

B, T, D_LF, D_DEC, D_KQ, D_CTX = 64, 2048, 512, 256, 128, 128
N_CORES = 8
EPC = B // N_CORES          # examples per core
P = 128
BATCH = 512                 # t-columns per inner batch
ALPHA = 0.2                 # leaky relu slope

# precision knobs (validated on HW: fp32r matmul relerr ~1.6e-4, fp32 ~2e-7)
KV_F32R = True              # key/value matmuls in fp32r (1 cyc/row) vs fp32 (4 cyc/row)
ENERGY_F32R = False         # energy matmul dtype (fp32 is cheap: 1 matmul/batch)


def _build_kernel(n_p: int):
    """Build the SPMD Bass program for packed length n_p (multiple of 512)."""
    assert n_p % BATCH == 0
    nb = n_p // BATCH
    nch = n_p // P

    kv_dt = F32R if KV_F32R else F32
    e_dt = F32R if ENERGY_F32R else F32

    nc = bacc.Bacc(None, target_bir_lowering=False)

    # ---- inputs (per-core content, same shapes everywhere) ----
    LFT = nc.dram_tensor("lft", [4, P, n_p], kv_dt, kind="ExternalInput")
    WKT = nc.dram_tensor("wkt", [4, P, D_KQ], kv_dt, kind="ExternalInput")
    WVT = nc.dram_tensor("wvt", [4, P, D_CTX], kv_dt, kind="ExternalInput")
    WQT = nc.dram_tensor("wqt", [2, P, D_KQ], F32, kind="ExternalInput")
    DECT = nc.dram_tensor("dect", [2, P, EPC], F32, kind="ExternalInput")
    BK = nc.dram_tensor("bk", [P, 1], F32, kind="ExternalInput")
    BV = nc.dram_tensor("bv", [P, 1], F32, kind="ExternalInput")
    BQ = nc.dram_tensor("bq", [P, 1], F32, kind="ExternalInput")
    MASK = nc.dram_tensor("mask", [EPC, n_p], F32, kind="ExternalInput")

    SCORE = nc.dram_tensor("score", [EPC, n_p], F32, kind="ExternalOutput")
    CTX = nc.dram_tensor("ctx", [EPC, D_CTX], F32, kind="ExternalOutput")

    with TileContext(nc) as tc:
        with (
            tc.tile_pool(name="const", bufs=1) as cpool,
            tc.tile_pool(name="big", bufs=1) as big,
            tc.tile_pool(name="io", bufs=3) as io,
            tc.tile_pool(name="kvp", bufs=3) as kvp,
            tc.tile_pool(name="ps", bufs=3, space="PSUM") as ps,
            tc.tile_pool(name="pse", bufs=2, space="PSUM") as pse,
            tc.tile_pool(name="pst", bufs=2, space="PSUM") as pst,
            tc.tile_pool(name="psc", bufs=1, space="PSUM") as psc,
        ):
            # ---- constants ----
            wkt = cpool.tile([P, 4, D_KQ], kv_dt, tag="wkt")
            nc.sync.dma_start(wkt[:], WKT.ap().rearrange("f p m -> p f m"))
            wvt = cpool.tile([P, 4, D_CTX], kv_dt, tag="wvt")
            nc.sync.dma_start(wvt[:], WVT.ap().rearrange("f p m -> p f m"))
            wqt = cpool.tile([P, 2, D_KQ], F32, tag="wqt")
            nc.sync.dma_start(wqt[:], WQT.ap().rearrange("f p m -> p f m"))
            dect = cpool.tile([P, 2, EPC], F32, tag="dect")
            nc.sync.dma_start(dect[:], DECT.ap().rearrange("f p m -> p f m"))
            bk = cpool.tile([P, 1], F32, tag="bk")
            nc.sync.dma_start(bk[:], BK[:, :])
            bv = cpool.tile([P, 1], F32, tag="bv")
            nc.sync.dma_start(bv[:], BV[:, :])
            bq = cpool.tile([P, 1], F32, tag="bq")
            nc.sync.dma_start(bq[:], BQ[:, :])
            mask = big.tile([EPC, n_p], F32, tag="mask")
            nc.sync.dma_start(mask[:], MASK[:, :])
            ident = cpool.tile([P, P], F32, tag="ident")
            make_identity(nc, ident[:])

            # ---- query: [k, ex] = lrelu(WqT.T-chunks @ decT + bq) ----
            psq = psc.tile([P, EPC], F32, tag="ctxq")
            nc.tensor.matmul(psq[:], wqt[:, 0], dect[:, 0], start=True, stop=False)
            nc.tensor.matmul(psq[:], wqt[:, 1], dect[:, 1], start=False, stop=True)
            queryT = cpool.tile([P, EPC], e_dt, tag="queryT")
            nc.scalar.activation(queryT[:], psq[:], AF.Prelu,
                                 bias=bq[:], scale=1.0, alpha=ALPHA)

            # ---- residents ----
            escore = big.tile([EPC, n_p], F32, tag="escore")    # exp -> masked -> score
            value_tc = big.tile([P, nch, D_CTX], F32, tag="value_tc")  # value in [t, c]
            sums = cpool.tile([EPC, 1], F32, tag="sums")
            recip = cpool.tile([EPC, 1], F32, tag="recip")
            psums = cpool.tile([EPC, nb], F32, tag="psums")
            ctx_ps = psc.tile([EPC, D_CTX], F32, tag="ctxq")

            # ---- main loop over 512-column batches ----
            for ib in range(nb):
                sl = slice(ib * BATCH, (ib + 1) * BATCH)
                lft = io.tile([P, 4, BATCH], kv_dt, tag="lft")
                nc.sync.dma_start(
                    lft[:], LFT.ap()[:, :, sl].rearrange("f p n -> p f n")
                )

                # keyT [k, 512]
                psk = ps.tile([P, BATCH], F32, tag="pskv")
                for fc in range(4):
                    nc.tensor.matmul(psk[:], wkt[:, fc], lft[:, fc],
                                     start=(fc == 0), stop=(fc == 3))
                keyT = kvp.tile([P, BATCH], e_dt, tag="keyT")
                nc.scalar.activation(keyT[:], psk[:], AF.Prelu,
                                     bias=bk[:], scale=1.0, alpha=ALPHA)

                # valueT [c, 512]
                psv = ps.tile([P, BATCH], F32, tag="pskv")
                for fc in range(4):
                    nc.tensor.matmul(psv[:], wvt[:, fc], lft[:, fc],
                                     start=(fc == 0), stop=(fc == 3))
                valueT = kvp.tile([P, BATCH], F32, tag="valueT")
                nc.scalar.activation(valueT[:], psv[:], AF.Prelu,
                                     bias=bv[:], scale=1.0, alpha=ALPHA)

                # energy [ex, 512] -> exp -> mask (exact 0s) -> per-batch row-sums
                pe_ = pse.tile([EPC, BATCH], F32, tag="pe")
                nc.tensor.matmul(pe_[:], queryT[:], keyT[:], start=True, stop=True)
                nc.scalar.activation(escore[:, sl], pe_[:], AF.Exp,
                                     bias=0.0, scale=1.0)
                nc.vector.tensor_tensor(escore[:, sl], escore[:, sl], mask[:, sl],
                                        mybir.AluOpType.mult)
                nc.vector.tensor_reduce(psums[:, ib:ib + 1], escore[:, sl],
                                        mybir.AxisListType.X, mybir.AluOpType.add)

                # value -> [t, c] chunks; scorenum -> [t, ex] chunks; context accum
                for j in range(4):
                    ic = ib * 4 + j
                    jsl = slice(j * P, (j + 1) * P)
                    pt = pst.tile([P, P], F32, tag="pt")
                    nc.tensor.transpose(pt[:], valueT[:, jsl], ident[:])
                    nc.scalar.copy(value_tc[:, ic], pt[:])

                    pst8 = pst.tile([P, EPC], F32, tag="pt")
                    nc.tensor.transpose(
                        pst8[:], escore[:, ib * BATCH + j * P:ib * BATCH + (j + 1) * P],
                        ident[:EPC, :EPC],
                    )
                    scoT = kvp.tile([P, EPC], F32, tag="scoT")
                    nc.vector.tensor_copy(scoT[:], pst8[:])
                    nc.tensor.matmul(ctx_ps[:], scoT[:], value_tc[:, ic],
                                     start=(ic == 0), stop=(ic == nch - 1))

            # ---- finalize ----
            nc.vector.tensor_reduce(sums[:], psums[:], mybir.AxisListType.X,
                                    mybir.AluOpType.add)
            nc.vector.reciprocal(recip[:], sums[:])
            nc.vector.tensor_scalar_mul(escore[:], escore[:], recip[:])
            nc.sync.dma_start(SCORE[:, :], escore[:])

            ctx_sb = cpool.tile([EPC, D_CTX], F32, tag="ctx_sb")
            nc.scalar.activation(ctx_sb[:], ctx_ps[:], AF.Copy,
                                 bias=0.0, scale=recip[:])
            nc.sync.dma_start(CTX[:, :], ctx_sb[:])

    nc.compile()
    return nc


def _pack_inputs(decoder_state, listener_feature, seq_sizes, Wq, bq, Wk, bk, Wv, bv):
    """Host-side packing: LPT-balance examples over cores, keep only valid
    (128-rounded) t-ranges, pre-transpose LF slices to [f, t] layout."""
    seq = np.asarray(seq_sizes).astype(np.int64)
    tiles = (seq + P - 1) // P                     # valid 128-tiles per example

    # LPT assignment: 8 bins, capacity EPC each, balance sum(tiles)
    order = np.argsort(-tiles, kind="stable")
    bins = [[] for _ in range(N_CORES)]
    loads = np.zeros(N_CORES, dtype=np.int64)
    for b_idx in order:
        open_bins = [c for c in range(N_CORES) if len(bins[c]) < EPC]
        c = min(open_bins, key=lambda c: loads[c])
        bins[c].append(int(b_idx))
        loads[c] += tiles[b_idx]

    n_p = int(loads.max()) * P
    n_p = max(BATCH, ((n_p + BATCH - 1) // BATCH) * BATCH)

    # shared weight layouts
    WkT = np.ascontiguousarray(Wk.T).reshape(4, P, D_KQ)     # [512,128]->[4,128,128]
    WvT = np.ascontiguousarray(Wv.T).reshape(4, P, D_CTX)
    WqT = np.ascontiguousarray(Wq.T).reshape(2, P, D_KQ)     # [256,128]->[2,128,128]
    bk_c = np.ascontiguousarray(bk.reshape(P, 1))
    bv_c = np.ascontiguousarray(bv.reshape(P, 1))
    bq_c = np.ascontiguousarray(bq.reshape(P, 1))

    in_maps, meta = [], []
    for c in range(N_CORES):
        lft = np.zeros((4, P, n_p), dtype=np.float32)
        msk = np.zeros((EPC, n_p), dtype=np.float32)
        dect = np.zeros((D_DEC, EPC), dtype=np.float32)
        starts = []
        pos = 0
        for e, b_idx in enumerate(bins[c]):
            rows = int(tiles[b_idx]) * P
            lf_t = listener_feature[b_idx, :rows, :].T      # [512, rows]
            lft[:, :, pos:pos + rows] = lf_t.reshape(4, P, rows)
            msk[e, pos:pos + int(seq[b_idx])] = 1.0
            dect[:, e] = decoder_state[b_idx]
            starts.append(pos)
            pos += rows
        in_maps.append({
            "lft": np.ascontiguousarray(lft),
            "wkt": WkT, "wvt": WvT, "wqt": WqT,
            "dect": np.ascontiguousarray(dect.reshape(2, P, EPC)),
            "bk": bk_c, "bv": bv_c, "bq": bq_c,
            "mask": msk,
        })
        meta.append((bins[c], starts))
    return in_maps, meta, n_p


def kernel(decoder_state, listener_feature, seq_sizes, Wq, bq, Wk, bk, Wv, bv,
           _trace=False):
    decoder_state = np.asarray(decoder_state, dtype=np.float32)
    listener_feature = np.asarray(listener_feature, dtype=np.float32)
    seq_sizes = np.asarray(seq_sizes)
    Wq = np.asarray(Wq, dtype=np.float32); bq = np.asarray(bq, dtype=np.float32)
    Wk = np.asarray(Wk, dtype=np.float32); bk = np.asarray(bk, dtype=np.float32)
    Wv = np.asarray(Wv, dtype=np.float32); bv = np.asarray(bv, dtype=np.float32)
    in_maps, meta, n_p = _pack_inputs(
        decoder_state, listener_feature, seq_sizes, Wq, bq, Wk, bk, Wv, bv)

    nc = _build_kernel(n_p)
    res = run_bass_kernel_spmd(nc, in_maps, core_ids=list(range(N_CORES)),
                               trace=_trace)

    seq = np.asarray(seq_sizes).astype(np.int64)
    att = np.zeros((B, T), dtype=np.float32)
    ctx = np.zeros((B, D_CTX), dtype=np.float32)
    for c in range(N_CORES):
        score_p = res.results[c]["score"]
        ctx_p = res.results[c]["ctx"]
        ex_ids, starts = meta[c]
        for e, b_idx in enumerate(ex_ids):
            s = int(seq[b_idx])
            att[b_idx, :s] = score_p[e, starts[e]:starts[e] + s]
            ctx[b_idx] = ctx_p[e]

    if _trace:
        kernel._last_results = res
    return att, ctx


# revision 6
# speedup vs baseline: 1.0936x; 1.0936x over previous
"""Trainium2 Bass kernel for nn_Attention (sparse attention with seq_sizes masking).

Computation (per example b over its own T axis):
    query = lrelu(dec @ Wq.T + bq)                        [B, 128]
    key   = lrelu(LF @ Wk.T + bk)                         [B, T, 128]
    energy= einsum('bk,btk->bt', query, key)              [B, T]
    att   = softmax(energy) * mask;  att /= sum(att)      [B, T]
    value = lrelu(LF @ Wv.T + bv)                         [B, T, 128]
    ctx   = einsum('bt,btc->bc', att, value)              [B, 128]

Strategy: data-parallel over B across 8 cores. Because att[t>=seq]==0 exactly
and the pre-mask softmax normalization cancels, rows t >= seq_sizes[b] never
matter. The host packs only the valid (128-rounded) row-range of each example
into a dense per-core buffer, pre-transposed to [f, t] layout so the f
contraction lands on SBUF partitions. Examples are LPT-balanced across cores
and slot-aligned (slot lengths = cross-core max) so per-example column ranges
are identical on every core -> one SPMD program with static APs.

Device inner loop per 512-column batch keeps the PE stream homogeneous
(all fp32r, N=512, 1 cyc/row): 4 keyT + 4 valueT accumulating matmuls,
1 energy matmul (queryT stationary), 1 ones(8x128) @ scorenum broadcast matmul
that collapses unnormalized scores across the example axis (exact: masked
entries are 0). Activations+bias fuse into ACT Prelu reads from PSUM; exp on
ACT; mask multiply on the otherwise-idle GpSimd; row-sums and the context
product+reduce (valueT * score_bcast, summed over each slot's static column
range) on DVE. Normalization by 1/rowsum folds into the final score scale and
the context PSUM->SBUF copy.
"""

import numpy as np

import concourse.bass as bass
import concourse.mybir as mybir
from concourse import bacc
from concourse.tile import TileContext
from concourse.masks import make_identity
from concourse.bass_utils import run_bass_kernel_spmd

F32 = mybir.dt.float32
F32R = mybir.dt.float32r
AF = mybir.ActivationFunctionType
ALU = mybir.AluOpType

B, T, D_LF, D_DEC, D_KQ, D_CTX = 64, 2048, 512, 256, 128, 128
N_CORES = 8
EPC = B // N_CORES          # examples (slots) per core
P = 128
BATCH = 512
ALPHA = 0.2                 # leaky relu slope


def _build_kernel(n_p: int, slot_starts, slot_ends):
    """SPMD program for packed length n_p; slot_starts/ends are the static
    per-example column ranges (identical across cores)."""
    assert n_p % BATCH == 0
    nb = n_p // BATCH

    nc = bacc.Bacc(None, target_bir_lowering=False)

    LFT = nc.dram_tensor("lft", [nb, P, 4 * BATCH], F32R, kind="ExternalInput")
    WKT = nc.dram_tensor("wkt", [4, P, D_KQ], F32R, kind="ExternalInput")
    WVT = nc.dram_tensor("wvt", [4, P, D_CTX], F32R, kind="ExternalInput")
    WQT = nc.dram_tensor("wqt", [2, P, D_KQ], F32, kind="ExternalInput")
    DECT = nc.dram_tensor("dect", [2, P, EPC], F32, kind="ExternalInput")
    BK = nc.dram_tensor("bk", [P, 1], F32, kind="ExternalInput")
    BV = nc.dram_tensor("bv", [P, 1], F32, kind="ExternalInput")
    BQ = nc.dram_tensor("bq", [P, 1], F32, kind="ExternalInput")
    MASK = nc.dram_tensor("mask", [EPC, n_p], F32, kind="ExternalInput")
    ONES8 = nc.dram_tensor("ones8", [EPC, P], F32R, kind="ExternalInput")

    SCORE = nc.dram_tensor("score", [EPC, n_p], F32R, kind="ExternalOutput")
    CTX = nc.dram_tensor("ctx", [EPC, D_CTX], F32, kind="ExternalOutput")

    # per-batch list of context sub-reduces; per-slot partial columns
    batch_parts = [[] for _ in range(nb)]
    slot_pcols = [[] for _ in range(EPC)]
    pcol = 0
    for e in range(EPC):
        s, t = int(slot_starts[e]), int(slot_ends[e])
        while s < t:
            ib = s // BATCH
            hi = min(t, (ib + 1) * BATCH)
            batch_parts[ib].append((pcol, e, s, hi))
            slot_pcols[e].append(pcol)
            pcol += 1
            s = hi
    n_pcols = pcol

    with TileContext(nc) as tc:
        with (
            tc.tile_pool(name="const", bufs=1) as cpool,
            tc.tile_pool(name="big", bufs=1) as big,
            tc.tile_pool(name="io", bufs=3) as io,
            tc.tile_pool(name="kvp", bufs=3) as kvp,
            tc.tile_pool(name="ps", bufs=3, space="PSUM") as ps,
            tc.tile_pool(name="pse", bufs=2, space="PSUM") as pse,
            tc.tile_pool(name="psb", bufs=2, space="PSUM") as psb,
            tc.tile_pool(name="psc", bufs=1, space="PSUM") as psc,
        ):
            # ---- constants ----
            wkt = cpool.tile([P, 4, D_KQ], F32R, tag="wkt")
            nc.sync.dma_start(wkt[:], WKT.ap().rearrange("f p m -> p f m"))
            wvt = cpool.tile([P, 4, D_CTX], F32R, tag="wvt")
            nc.sync.dma_start(wvt[:], WVT.ap().rearrange("f p m -> p f m"))
            wqt = cpool.tile([P, 2, D_KQ], F32, tag="wqt")
            nc.sync.dma_start(wqt[:], WQT.ap().rearrange("f p m -> p f m"))
            dect = cpool.tile([P, 2, EPC], F32, tag="dect")
            nc.sync.dma_start(dect[:], DECT.ap().rearrange("f p m -> p f m"))
            bk = cpool.tile([P, 1], F32, tag="bk")
            nc.sync.dma_start(bk[:], BK[:, :])
            bv = cpool.tile([P, 1], F32, tag="bv")
            nc.sync.dma_start(bv[:], BV[:, :])
            bq = cpool.tile([P, 1], F32, tag="bq")
            nc.sync.dma_start(bq[:], BQ[:, :])
            mask = big.tile([EPC, n_p], F32, tag="mask")
            nc.sync.dma_start(mask[:], MASK[:, :])
            ones8 = cpool.tile([EPC, P], F32R, tag="ones8")
            nc.sync.dma_start(ones8[:], ONES8[:, :])
            ident = cpool.tile([P, P], F32, tag="ident")
            make_identity(nc, ident[:])

            # ---- query: [k, ex] = lrelu(WqT-chunks.T @ decT + bq), fp32r out ----
            psq = psc.tile([P, EPC], F32, tag="ctxq")
            nc.tensor.matmul(psq[:], wqt[:, 0], dect[:, 0], start=True, stop=False)
            nc.tensor.matmul(psq[:], wqt[:, 1], dect[:, 1], start=False, stop=True)
            queryT = cpool.tile([P, EPC], F32R, tag="queryT")
            nc.scalar.activation(queryT[:], psq[:], AF.Prelu,
                                 bias=bq[:], scale=1.0, alpha=ALPHA)

            # ---- residents ----
            escore = big.tile([EPC, n_p], F32R, tag="escore")
            psums = cpool.tile([EPC, nb], F32, tag="psums")
            sums = cpool.tile([EPC, 1], F32, tag="sums")
            recip = cpool.tile([EPC, 1], F32, tag="recip")
            ctx_part = big.tile([P, max(n_pcols, 1)], F32, tag="ctx_part")
            ctx_cols = cpool.tile([P, EPC], F32, tag="ctx_cols")

            # ---- main loop ----
            for ib in range(nb):
                sl = slice(ib * BATCH, (ib + 1) * BATCH)
                lft = io.tile([P, 4, BATCH], F32R, tag="lft")
                nc.sync.dma_start(
                    lft[:], LFT.ap()[ib].rearrange("p (f n) -> p f n", f=4))

                psk = ps.tile([P, BATCH], F32, tag="pskv")
                for fc in range(4):
                    nc.tensor.matmul(psk[:], wkt[:, fc], lft[:, fc],
                                     start=(fc == 0), stop=(fc == 3))
                keyT = kvp.tile([P, BATCH], F32R, tag="keyT")
                nc.scalar.activation(keyT[:], psk[:], AF.Prelu,
                                     bias=bk[:], scale=1.0, alpha=ALPHA)

                psv = ps.tile([P, BATCH], F32, tag="pskv")
                for fc in range(4):
                    nc.tensor.matmul(psv[:], wvt[:, fc], lft[:, fc],
                                     start=(fc == 0), stop=(fc == 3))
                valueT = kvp.tile([P, BATCH], F32, tag="valueT")
                nc.scalar.activation(valueT[:], psv[:], AF.Prelu,
                                     bias=bv[:], scale=1.0, alpha=ALPHA)

                # energy -> exp -> mask (GpSimd) -> row-sum partials (DVE)
                pe_ = pse.tile([EPC, BATCH], F32, tag="pe")
                nc.tensor.matmul(pe_[:], queryT[:], keyT[:], start=True, stop=True)
                nc.scalar.activation(escore[:, sl], pe_[:], AF.Exp,
                                     bias=0.0, scale=1.0)
                nc.gpsimd.tensor_tensor(escore[:, sl], escore[:, sl], mask[:, sl],
                                        ALU.mult)
                nc.vector.tensor_reduce(psums[:, ib:ib + 1], escore[:, sl],
                                        mybir.AxisListType.X, ALU.add)

                # score broadcast across partitions (masked cols are exact 0,
                # so the column-sum over examples recovers the owner's score)
                psbt = psb.tile([P, BATCH], F32, tag="sb")
                nc.tensor.matmul(psbt[:], ones8[:], escore[:, sl],
                                 start=True, stop=True)

                # context partials: prod = valueT * score_bcast; reduce slots
                prod = kvp.tile([P, BATCH], F32, tag="prod")
                nc.vector.tensor_tensor(prod[:], valueT[:], psbt[:], ALU.mult)
                for (pc, e, lo, hi) in batch_parts[ib]:
                    nc.vector.tensor_reduce(
                        ctx_part[:, pc:pc + 1],
                        prod[:, lo - ib * BATCH:hi - ib * BATCH],
                        mybir.AxisListType.X, ALU.add)

            # ---- finalize ----
            nc.vector.tensor_reduce(sums[:], psums[:], mybir.AxisListType.X,
                                    ALU.add)
            nc.vector.reciprocal(recip[:], sums[:])
            nc.vector.tensor_scalar_mul(escore[:], escore[:], recip[:])
            nc.sync.dma_start(SCORE[:, :], escore[:])

            for e in range(EPC):
                pcs = slot_pcols[e]
                if len(pcs) == 1:
                    nc.vector.tensor_copy(ctx_cols[:, e:e + 1],
                                          ctx_part[:, pcs[0]:pcs[0] + 1])
                else:
                    assert pcs == list(range(pcs[0], pcs[-1] + 1))
                    nc.vector.tensor_reduce(
                        ctx_cols[:, e:e + 1],
                        ctx_part[:, pcs[0]:pcs[-1] + 1],
                        mybir.AxisListType.X, ALU.add)

            ctx_ps = psc.tile([EPC, D_CTX], F32, tag="ctxq")
            nc.tensor.transpose(ctx_ps[:], ctx_cols[:], ident[:])
            ctx_sb = cpool.tile([EPC, D_CTX], F32, tag="ctx_sb")
            nc.scalar.activation(ctx_sb[:], ctx_ps[:], AF.Copy,
                                 bias=0.0, scale=recip[:])
            nc.sync.dma_start(CTX[:, :], ctx_sb[:])

    nc.compile()
    return nc


def _pack_inputs(decoder_state, listener_feature, seq_sizes, Wq, bq, Wk, bk, Wv, bv):
    """LPT-balance examples over cores; slot-align (cross-core max slot
    lengths); pre-transpose LF to [f, t] in a batch-local layout."""
    seq = np.asarray(seq_sizes).astype(np.int64)
    tiles = (seq + P - 1) // P

    order = np.argsort(-tiles, kind="stable")
    bins = [[] for _ in range(N_CORES)]
    loads = np.zeros(N_CORES, dtype=np.int64)
    for b_idx in order:
        open_bins = [c for c in range(N_CORES) if len(bins[c]) < EPC]
        c = min(open_bins, key=lambda c: loads[c])
        bins[c].append(int(b_idx))
        loads[c] += tiles[b_idx]
    # slot-align: per core sort desc, slot length = max over cores
    for c in range(N_CORES):
        bins[c].sort(key=lambda b_idx: -tiles[b_idx])
    slot_len = np.zeros(EPC, dtype=np.int64)
    for c in range(N_CORES):
        for e, b_idx in enumerate(bins[c]):
            slot_len[e] = max(slot_len[e], tiles[b_idx])
    slot_rows = slot_len * P
    n_p = int(slot_rows.sum())
    n_p = max(BATCH, ((n_p + BATCH - 1) // BATCH) * BATCH)
    slot_starts = np.concatenate([[0], np.cumsum(slot_rows)])[:EPC]
    slot_ends = slot_starts + slot_rows
    nb = n_p // BATCH

    WkT = np.ascontiguousarray(Wk.T).reshape(4, P, D_KQ)
    WvT = np.ascontiguousarray(Wv.T).reshape(4, P, D_CTX)
    WqT = np.ascontiguousarray(Wq.T).reshape(2, P, D_KQ)
    bk_c = np.ascontiguousarray(bk.reshape(P, 1))
    bv_c = np.ascontiguousarray(bv.reshape(P, 1))
    bq_c = np.ascontiguousarray(bq.reshape(P, 1))
    ones8 = np.ones((EPC, P), dtype=np.float32)

    in_maps, meta = [], []
    for c in range(N_CORES):
        lft = np.zeros((P, 4, n_p), dtype=np.float32)
        msk = np.zeros((EPC, n_p), dtype=np.float32)
        dect = np.zeros((D_DEC, EPC), dtype=np.float32)
        for e, b_idx in enumerate(bins[c]):
            pos = int(slot_starts[e])
            rows = int(tiles[b_idx]) * P
            lf_t = listener_feature[b_idx, :rows, :].T      # [512, rows]
            lft[:, :, pos:pos + rows] = np.transpose(
                lf_t.reshape(4, P, rows), (1, 0, 2))
            msk[e, pos:pos + int(seq[b_idx])] = 1.0
            dect[:, e] = decoder_state[b_idx]
        # batch-local layout: [nb, P, 4*BATCH], per partition contiguous
        lft_b = np.transpose(lft.reshape(P, 4, nb, BATCH), (2, 0, 1, 3))
        in_maps.append({
            "lft": np.ascontiguousarray(lft_b).reshape(nb, P, 4 * BATCH),
            "wkt": WkT, "wvt": WvT, "wqt": WqT,
            "dect": np.ascontiguousarray(dect.reshape(2, P, EPC)),
            "bk": bk_c, "bv": bv_c, "bq": bq_c,
            "mask": msk, "ones8": ones8,
        })
        meta.append(bins[c])
    return in_maps, meta, n_p, slot_starts, slot_ends


def kernel(decoder_state, listener_feature, seq_sizes, Wq, bq, Wk, bk, Wv, bv,
           _trace=False):
    decoder_state = np.asarray(decoder_state, dtype=np.float32)
    listener_feature = np.asarray(listener_feature, dtype=np.float32)
    seq_sizes = np.asarray(seq_sizes)
    Wq = np.asarray(Wq, dtype=np.float32); bq = np.asarray(bq, dtype=np.float32)
    Wk = np.asarray(Wk, dtype=np.float32); bk = np.asarray(bk, dtype=np.float32)
    Wv = np.asarray(Wv, dtype=np.float32); bv = np.asarray(bv, dtype=np.float32)
    in_maps, meta, n_p, slot_starts, slot_ends = _pack_inputs(
        decoder_state, listener_feature, seq_sizes, Wq, bq, Wk, bk, Wv, bv)

    nc = _build_kernel(n_p, slot_starts, slot_ends)
    res = run_bass_kernel_spmd(nc, in_maps, core_ids=list(range(N_CORES)),
                               trace=_trace)

    seq = np.asarray(seq_sizes).astype(np.int64)
    att = np.zeros((B, T), dtype=np.float32)
    ctx = np.zeros((B, D_CTX), dtype=np.float32)
    for c in range(N_CORES):
        score_p = res.results[c]["score"]
        ctx_p = res.results[c]["ctx"]
        for e, b_idx in enumerate(meta[c]):
            s = int(seq[b_idx])
            st = int(slot_starts[e])
            att[b_idx, :s] = score_p[e, st:st + s]
            ctx[b_idx] = ctx_p[e]

    if _trace:
        kernel._last_results = res
    return att, ctx
